# revision 2
# baseline (speedup 1.0000x reference)
"""Trainium2 Bass kernel v6 for CubicalLayer gather_nd.

Problem: X[4096,4096] f32, indices[524288,2] int32 ->
         out[262144,2] f32, out.flat[k] = X[indices[k,0], indices[k,1]].

Strategy (8 NeuronCores, row-striped; ~65536 requests per core):
  - Host shards pairs by row stripe (core = r>>9) and phase class
    (ph = c%64, 64 classes padded to CLS=1152 slots) and precomputes the
    int16 256B-block index blk = (r%512)*64 + c//64 for each slot (the
    device does no index arithmetic; only the 1.2MB int16 index array is
    uploaded per pass instead of 4.7MB of int32 pairs).
  - Within a class, requests stay in natural order (measured on HW:
    sorted/rotated/interleaved orders are no faster than random for this
    SDMA gather -- it is descriptor-rate bound, not DRAM-row bound) and
    padding slots gather spread dummy blocks (recycled-duplicate padding
    concentrates addresses and measurably slows the gather).
  - Device: 72 SWDGE dma_gather ops per pass (1024 indices each -- the Q7
    idx-scratch limit -- 256B blocks, round-robin over 4 queues), with
    single_packet=False so the SDMA engines batch many descriptors per
    packet instead of context-switching every 256B (measured ~20% faster);
    the vector engine extracts each class's phase element with static
    strided copies.
  - Host unshards: scatters per-core results back to original pair order.
"""

import numpy as np

import concourse.tile as tile
from concourse import bacc, mybir
from concourse.bass_utils import run_bass_kernel_spmd

H = 4096
W = 4096
N_IDX = 524288
NCORES = 8
P = 128

STRIPE_ROWS = H // NCORES  # 512
ELEM = 64  # f32 per gathered block (256B)
NPH = 64  # phase classes (c % 64)
CLS = 1152  # padded slots per class (9*128; binomial mean 1024, +4 sigma)
NPAD = NPH * CLS  # 73728
GCHUNK = 1024  # indices per dma_gather (HW SWDGE ring limit)
NQ = 4  # SWDGE queues
NCHUNKS = NPAD // GCHUNK  # 72
COLS = NPAD // P  # 576
NBLK = STRIPE_ROWS * W // ELEM  # 32768 blocks per stripe
IDXW = NPAD // 16  # 4608


def build_kernel(reps=1):
    cg = GCHUNK // P  # 8 groups per chunk
    cls_g = CLS // P  # 9 groups per class

    nc = bacc.Bacc(
        "TRN2",
        target_bir_lowering=False,
        debug=False,
        num_devices=NCORES,
        num_swdge_queues=NQ,
    )
    XS = nc.dram_tensor("XS", [STRIPE_ROWS, W], mybir.dt.float32, kind="ExternalInput")
    IDX = nc.dram_tensor("IDX", [P, IDXW], mybir.dt.int16, kind="ExternalInput")
    OUT = nc.dram_tensor("OUT", [P, COLS], mybir.dt.float32, kind="ExternalOutput")

    xs_rows = XS.ap().rearrange("h (a b) -> (h a) b", b=ELEM)  # [32768, 64]

    with tile.TileContext(nc) as tc:
        with (
            tc.tile_pool(name="ip", bufs=2) as i_pool,
            tc.tile_pool(name="gp", bufs=8) as g_pool,
            tc.tile_pool(name="op", bufs=1) as o_pool,
        ):
            vals = o_pool.tile([P, COLS], mybir.dt.float32)

            with tc.For_i(0, reps, 1):
                idx_sb = i_pool.tile([P, IDXW], mybir.dt.int16, tag="idx")
                nc.sync.dma_start(out=idx_sb[:, :], in_=IDX.ap())
                for c in range(NCHUNKS):
                    gsl = slice(c * (GCHUNK // 16), (c + 1) * (GCHUNK // 16))
                    g = g_pool.tile([P, cg, ELEM], mybir.dt.float32, tag="g")
                    nc.gpsimd.dma_gather(
                        out_ap=g[:, :, :],
                        in_ap=xs_rows,
                        idxs_ap=idx_sb[:, gsl],
                        num_idxs=GCHUNK,
                        num_idxs_reg=GCHUNK,
                        elem_size=ELEM,
                        queue_num=c % NQ,
                        single_packet=False,
                    )
                    # extract the phase element: the chunk's group range
                    # [8c, 8c+8) intersects classes (9 groups each) at
                    # static boundaries
                    g_lo = c * cg
                    while g_lo < (c + 1) * cg:
                        cls_idx = g_lo // cls_g
                        g_hi = min((cls_idx + 1) * cls_g, (c + 1) * cg)
                        nc.vector.tensor_copy(
                            out=vals[:, g_lo:g_hi],
                            in_=g[:, g_lo - c * cg : g_hi - c * cg, cls_idx],
                        )
                        g_lo = g_hi

            nc.sync.dma_start(out=OUT.ap(), in_=vals[:, :])
    nc.compile()
    return nc


_NC_CACHE = {}


def _get_nc():
    if "nc" not in _NC_CACHE:
        _NC_CACHE["nc"] = build_kernel()
    return _NC_CACHE["nc"]


def _route(indices):
    """Host-side shard: route pairs to (core, class) slots, build the int16
    block-index array per core and the slot->pair map."""
    r = indices[:, 0].astype(np.int64)
    c = indices[:, 1].astype(np.int64)
    core = r >> 9
    ph = c & (NPH - 1)
    blk = ((r & (STRIPE_ROWS - 1)) << 6) | (c >> 6)  # [0, 32768)
    key = core * NPH + ph  # 512 bins
    order = np.argsort(key, kind="stable")
    counts = np.bincount(key, minlength=NCORES * NPH)
    assert counts.max() <= CLS, f"class count {counts.max()} exceeds CLS={CLS}"
    starts = np.concatenate([[0], np.cumsum(counts)])

    # padding slots gather spread dummy blocks (uniform over the stripe);
    # measured: natural request order beats sorted/interleaved orders, and
    # recycled-duplicate padding concentrates addresses and slows SDMA
    pad_fill = ((np.arange(NPAD, dtype=np.int64) * 9973) & (NBLK - 1)).astype(np.int16)

    in_maps = []
    gather_pos = []
    for i in range(NCORES):
        idx_arr = pad_fill.copy()
        slot_all = []
        rows_all = []
        for p in range(NPH):
            b = i * NPH + p
            seg = order[starts[b] : starts[b + 1]]  # pair rows
            n = len(seg)
            base = p * CLS
            if n == 0:
                continue
            idx_arr[base : base + n] = blk[seg].astype(np.int16)
            slot_all.append(base + np.arange(n))
            rows_all.append(seg)
        iw = np.tile(idx_arr.reshape(NPAD // 16, 16).T, (8, 1))
        in_maps.append({"IDX": iw})
        gather_pos.append((np.concatenate(slot_all), np.concatenate(rows_all)))
    return in_maps, gather_pos


def kernel(X, indices):
    X = np.ascontiguousarray(np.asarray(X), dtype=np.float32)
    indices = np.asarray(indices, dtype=np.int32)
    nc = _get_nc()
    in_maps, gather_pos = _route(indices)
    for i in range(NCORES):
        in_maps[i]["XS"] = np.ascontiguousarray(
            X[i * STRIPE_ROWS : (i + 1) * STRIPE_ROWS]
        )
    res = run_bass_kernel_spmd(nc, in_maps, core_ids=list(range(NCORES)))
    out_flat = np.empty(N_IDX, np.float32)
    k = np.arange(NPAD)
    # routed slot k -> flat position in [P, COLS]: vals[k%128, k//128]
    land = (k % P) * COLS + k // P
    for i in range(NCORES):
        vals = res.results[i]["OUT"].reshape(-1)
        slots, rows = gather_pos[i]
        out_flat[rows] = vals[land[slots]]
    return out_flat.reshape(-1, 2)



# revision 3
# speedup vs baseline: 1.0758x; 1.0758x over previous
"""Trainium2 Bass kernel v6 for CubicalLayer gather_nd.

Problem: X[4096,4096] f32, indices[524288,2] int32 ->
         out[262144,2] f32, out.flat[k] = X[indices[k,0], indices[k,1]].

Strategy (8 NeuronCores, row-striped; ~65536 requests per core):
  - Host shards pairs by row stripe (core = r>>9) and phase class
    (ph = c%64, 64 classes padded to CLS=1152 slots) and precomputes the
    int16 256B-block index blk = (r%512)*64 + c//64 for each slot (the
    device does no index arithmetic; only the 1.2MB int16 index array is
    uploaded per pass instead of 4.7MB of int32 pairs).
  - Within a class, requests stay in natural order (measured on HW:
    sorted/rotated/interleaved orders are no faster than random for this
    SDMA gather -- it is descriptor-rate bound, not DRAM-row bound) and
    padding slots gather spread dummy blocks (recycled-duplicate padding
    concentrates addresses and measurably slows the gather).
  - Device: 48 SWDGE dma_gather ops per pass (1536 indices each -- 6KB of
    int32 fits the Q7 idx scratch, 2048 does not; descriptor ring carveout
    doubled to 2048 entries -- 256B blocks, round-robin over 4 queues), with
    single_packet=False so the SDMA engines batch many descriptors per
    packet instead of context-switching every 256B (measured ~20% faster);
    the vector engine extracts each class's phase element with static
    strided copies.
  - Host unshards: scatters per-core results back to original pair order.
"""

import numpy as np

import concourse.tile as tile
from concourse import bacc, mybir
from concourse.bass_utils import run_bass_kernel_spmd

H = 4096
W = 4096
N_IDX = 524288
NCORES = 8
P = 128

STRIPE_ROWS = H // NCORES  # 512
ELEM = 64  # f32 per gathered block (256B)
NPH = 64  # phase classes (c % 64)
CLS = 1152  # padded slots per class (9*128; binomial mean 1024, +4 sigma)
NPAD = NPH * CLS  # 73728
GCHUNK = 1536  # indices per dma_gather (Q7 idx-scratch fits 6KB)
NQ = 4  # SWDGE queues
NCHUNKS = NPAD // GCHUNK  # 72
COLS = NPAD // P  # 576
NBLK = STRIPE_ROWS * W // ELEM  # 32768 blocks per stripe
IDXW = NPAD // 16  # 4608


def build_kernel(reps=1):
    cg = GCHUNK // P  # 8 groups per chunk
    cls_g = CLS // P  # 9 groups per class

    nc = bacc.Bacc(
        "TRN2",
        target_bir_lowering=False,
        debug=False,
        num_devices=NCORES,
        num_swdge_queues=NQ,
        dynamic_dma_scratch_size=32768,
    )
    XS = nc.dram_tensor("XS", [STRIPE_ROWS, W], mybir.dt.float32, kind="ExternalInput")
    IDX = nc.dram_tensor("IDX", [P, IDXW], mybir.dt.int16, kind="ExternalInput")
    OUT = nc.dram_tensor("OUT", [P, COLS], mybir.dt.float32, kind="ExternalOutput")

    xs_rows = XS.ap().rearrange("h (a b) -> (h a) b", b=ELEM)  # [32768, 64]

    with tile.TileContext(nc) as tc:
        with (
            tc.tile_pool(name="ip", bufs=2) as i_pool,
            tc.tile_pool(name="gp", bufs=8) as g_pool,
            tc.tile_pool(name="op", bufs=1) as o_pool,
        ):
            vals = o_pool.tile([P, COLS], mybir.dt.float32)

            with tc.For_i(0, reps, 1):
                idx_sb = i_pool.tile([P, IDXW], mybir.dt.int16, tag="idx")
                nc.sync.dma_start(out=idx_sb[:, :], in_=IDX.ap())
                for c in range(NCHUNKS):
                    gsl = slice(c * (GCHUNK // 16), (c + 1) * (GCHUNK // 16))
                    g = g_pool.tile([P, cg, ELEM], mybir.dt.float32, tag="g")
                    nc.gpsimd.dma_gather(
                        out_ap=g[:, :, :],
                        in_ap=xs_rows,
                        idxs_ap=idx_sb[:, gsl],
                        num_idxs=GCHUNK,
                        num_idxs_reg=GCHUNK,
                        elem_size=ELEM,
                        queue_num=c % NQ,
                        single_packet=False,
                    )
                    # extract the phase element: the chunk's group range
                    # [8c, 8c+8) intersects classes (9 groups each) at
                    # static boundaries
                    g_lo = c * cg
                    while g_lo < (c + 1) * cg:
                        cls_idx = g_lo // cls_g
                        g_hi = min((cls_idx + 1) * cls_g, (c + 1) * cg)
                        nc.vector.tensor_copy(
                            out=vals[:, g_lo:g_hi],
                            in_=g[:, g_lo - c * cg : g_hi - c * cg, cls_idx],
                        )
                        g_lo = g_hi

            nc.sync.dma_start(out=OUT.ap(), in_=vals[:, :])
    nc.compile()
    return nc


_NC_CACHE = {}


def _get_nc():
    if "nc" not in _NC_CACHE:
        _NC_CACHE["nc"] = build_kernel()
    return _NC_CACHE["nc"]


def _route(indices):
    """Host-side shard: route pairs to (core, class) slots, build the int16
    block-index array per core and the slot->pair map."""
    r = indices[:, 0].astype(np.int64)
    c = indices[:, 1].astype(np.int64)
    core = r >> 9
    ph = c & (NPH - 1)
    blk = ((r & (STRIPE_ROWS - 1)) << 6) | (c >> 6)  # [0, 32768)
    key = core * NPH + ph  # 512 bins
    order = np.argsort(key, kind="stable")
    counts = np.bincount(key, minlength=NCORES * NPH)
    assert counts.max() <= CLS, f"class count {counts.max()} exceeds CLS={CLS}"
    starts = np.concatenate([[0], np.cumsum(counts)])

    # padding slots gather spread dummy blocks (uniform over the stripe);
    # measured: natural request order beats sorted/interleaved orders, and
    # recycled-duplicate padding concentrates addresses and slows SDMA
    pad_fill = ((np.arange(NPAD, dtype=np.int64) * 9973) & (NBLK - 1)).astype(np.int16)

    in_maps = []
    gather_pos = []
    for i in range(NCORES):
        idx_arr = pad_fill.copy()
        slot_all = []
        rows_all = []
        for p in range(NPH):
            b = i * NPH + p
            seg = order[starts[b] : starts[b + 1]]  # pair rows
            n = len(seg)
            base = p * CLS
            if n == 0:
                continue
            idx_arr[base : base + n] = blk[seg].astype(np.int16)
            slot_all.append(base + np.arange(n))
            rows_all.append(seg)
        iw = np.tile(idx_arr.reshape(NPAD // 16, 16).T, (8, 1))
        in_maps.append({"IDX": iw})
        gather_pos.append((np.concatenate(slot_all), np.concatenate(rows_all)))
    return in_maps, gather_pos


def kernel(X, indices):
    X = np.ascontiguousarray(np.asarray(X), dtype=np.float32)
    indices = np.asarray(indices, dtype=np.int32)
    nc = _get_nc()
    in_maps, gather_pos = _route(indices)
    for i in range(NCORES):
        in_maps[i]["XS"] = np.ascontiguousarray(
            X[i * STRIPE_ROWS : (i + 1) * STRIPE_ROWS]
        )
    res = run_bass_kernel_spmd(nc, in_maps, core_ids=list(range(NCORES)))
    out_flat = np.empty(N_IDX, np.float32)
    k = np.arange(NPAD)
    # routed slot k -> flat position in [P, COLS]: vals[k%128, k//128]
    land = (k % P) * COLS + k // P
    for i in range(NCORES):
        vals = res.results[i]["OUT"].reshape(-1)
        slots, rows = gather_pos[i]
        out_flat[rows] = vals[land[slots]]
    return out_flat.reshape(-1, 2)



# revision 4
# speedup vs baseline: 1.1047x; 1.0268x over previous
"""Trainium2 Bass kernel v6 for CubicalLayer gather_nd.

Problem: X[4096,4096] f32, indices[524288,2] int32 ->
         out[262144,2] f32, out.flat[k] = X[indices[k,0], indices[k,1]].

Strategy (8 NeuronCores, row-striped; ~65536 requests per core):
  - Host shards pairs by row stripe (core = r>>9) and phase class
    (ph = c%64, 64 classes padded to CLS=1152 slots) and precomputes the
    int16 256B-block index blk = (r%512)*64 + c//64 for each slot (the
    device does no index arithmetic; only the 1.2MB int16 index array is
    uploaded per pass instead of 4.7MB of int32 pairs).
  - Within a class, requests stay in natural order (measured on HW:
    sorted/rotated/interleaved orders are no faster than random for this
    SDMA gather -- it is descriptor-rate bound, not DRAM-row bound) and
    padding slots gather spread dummy blocks (recycled-duplicate padding
    concentrates addresses and measurably slows the gather).
  - Device: 32 SWDGE dma_gather ops per pass (2304 indices each; the Q7
    data scratch is 64KB so the int32 idx copy fits easily -- the old 1024
    "ring limit" was a chunk==ring-size wrap bug, so the descriptor ring
    carveout is raised to 4096 entries and chunks stay below it -- 256B
    blocks, round-robin over 4 queues), with
    single_packet=False so the SDMA engines batch many descriptors per
    packet instead of context-switching every 256B (measured ~20% faster);
    the vector engine extracts each class's phase element with static
    strided copies.
  - Host unshards: scatters per-core results back to original pair order.
"""

import numpy as np

import concourse.tile as tile
from concourse import bacc, mybir
from concourse.bass_utils import run_bass_kernel_spmd

H = 4096
W = 4096
N_IDX = 524288
NCORES = 8
P = 128

STRIPE_ROWS = H // NCORES  # 512
ELEM = 64  # f32 per gathered block (256B)
NPH = 64  # phase classes (c % 64)
CLS = 1152  # padded slots per class (9*128; binomial mean 1024, +4 sigma)
NPAD = NPH * CLS  # 73728
GCHUNK = 2304  # indices per dma_gather
NQ = 4  # SWDGE queues
NCHUNKS = NPAD // GCHUNK  # 72
COLS = NPAD // P  # 576
NBLK = STRIPE_ROWS * W // ELEM  # 32768 blocks per stripe
IDXW = NPAD // 16  # 4608


def build_kernel(reps=1):
    cg = GCHUNK // P  # 8 groups per chunk
    cls_g = CLS // P  # 9 groups per class

    nc = bacc.Bacc(
        "TRN2",
        target_bir_lowering=False,
        debug=False,
        num_devices=NCORES,
        num_swdge_queues=NQ,
        dynamic_dma_scratch_size=65536,
    )
    XS = nc.dram_tensor("XS", [STRIPE_ROWS, W], mybir.dt.float32, kind="ExternalInput")
    IDX = nc.dram_tensor("IDX", [P, IDXW], mybir.dt.int16, kind="ExternalInput")
    OUT = nc.dram_tensor("OUT", [P, COLS], mybir.dt.float32, kind="ExternalOutput")

    xs_rows = XS.ap().rearrange("h (a b) -> (h a) b", b=ELEM)  # [32768, 64]

    with tile.TileContext(nc) as tc:
        with (
            tc.tile_pool(name="ip", bufs=2) as i_pool,
            tc.tile_pool(name="gp", bufs=8) as g_pool,
            tc.tile_pool(name="op", bufs=1) as o_pool,
        ):
            vals = o_pool.tile([P, COLS], mybir.dt.float32)

            with tc.For_i(0, reps, 1):
                idx_sb = i_pool.tile([P, IDXW], mybir.dt.int16, tag="idx")
                nc.sync.dma_start(out=idx_sb[:, :], in_=IDX.ap())
                for c in range(NCHUNKS):
                    gsl = slice(c * (GCHUNK // 16), (c + 1) * (GCHUNK // 16))
                    g = g_pool.tile([P, cg, ELEM], mybir.dt.float32, tag="g")
                    nc.gpsimd.dma_gather(
                        out_ap=g[:, :, :],
                        in_ap=xs_rows,
                        idxs_ap=idx_sb[:, gsl],
                        num_idxs=GCHUNK,
                        num_idxs_reg=GCHUNK,
                        elem_size=ELEM,
                        queue_num=c % NQ,
                        single_packet=False,
                    )
                    # extract the phase element: the chunk's group range
                    # [8c, 8c+8) intersects classes (9 groups each) at
                    # static boundaries
                    g_lo = c * cg
                    while g_lo < (c + 1) * cg:
                        cls_idx = g_lo // cls_g
                        g_hi = min((cls_idx + 1) * cls_g, (c + 1) * cg)
                        nc.vector.tensor_copy(
                            out=vals[:, g_lo:g_hi],
                            in_=g[:, g_lo - c * cg : g_hi - c * cg, cls_idx],
                        )
                        g_lo = g_hi

            nc.sync.dma_start(out=OUT.ap(), in_=vals[:, :])
    nc.compile()
    return nc


_NC_CACHE = {}


def _get_nc():
    if "nc" not in _NC_CACHE:
        _NC_CACHE["nc"] = build_kernel()
    return _NC_CACHE["nc"]


def _route(indices):
    """Host-side shard: route pairs to (core, class) slots, build the int16
    block-index array per core and the slot->pair map."""
    r = indices[:, 0].astype(np.int64)
    c = indices[:, 1].astype(np.int64)
    core = r >> 9
    ph = c & (NPH - 1)
    blk = ((r & (STRIPE_ROWS - 1)) << 6) | (c >> 6)  # [0, 32768)
    key = core * NPH + ph  # 512 bins
    order = np.argsort(key, kind="stable")
    counts = np.bincount(key, minlength=NCORES * NPH)
    assert counts.max() <= CLS, f"class count {counts.max()} exceeds CLS={CLS}"
    starts = np.concatenate([[0], np.cumsum(counts)])

    # padding slots gather spread dummy blocks (uniform over the stripe);
    # measured: natural request order beats sorted/interleaved orders, and
    # recycled-duplicate padding concentrates addresses and slows SDMA
    pad_fill = ((np.arange(NPAD, dtype=np.int64) * 9973) & (NBLK - 1)).astype(np.int16)

    in_maps = []
    gather_pos = []
    for i in range(NCORES):
        idx_arr = pad_fill.copy()
        slot_all = []
        rows_all = []
        for p in range(NPH):
            b = i * NPH + p
            seg = order[starts[b] : starts[b + 1]]  # pair rows
            n = len(seg)
            base = p * CLS
            if n == 0:
                continue
            idx_arr[base : base + n] = blk[seg].astype(np.int16)
            slot_all.append(base + np.arange(n))
            rows_all.append(seg)
        iw = np.tile(idx_arr.reshape(NPAD // 16, 16).T, (8, 1))
        in_maps.append({"IDX": iw})
        gather_pos.append((np.concatenate(slot_all), np.concatenate(rows_all)))
    return in_maps, gather_pos


def kernel(X, indices):
    X = np.ascontiguousarray(np.asarray(X), dtype=np.float32)
    indices = np.asarray(indices, dtype=np.int32)
    nc = _get_nc()
    in_maps, gather_pos = _route(indices)
    for i in range(NCORES):
        in_maps[i]["XS"] = np.ascontiguousarray(
            X[i * STRIPE_ROWS : (i + 1) * STRIPE_ROWS]
        )
    res = run_bass_kernel_spmd(nc, in_maps, core_ids=list(range(NCORES)))
    out_flat = np.empty(N_IDX, np.float32)
    k = np.arange(NPAD)
    # routed slot k -> flat position in [P, COLS]: vals[k%128, k//128]
    land = (k % P) * COLS + k // P
    for i in range(NCORES):
        vals = res.results[i]["OUT"].reshape(-1)
        slots, rows = gather_pos[i]
        out_flat[rows] = vals[land[slots]]
    return out_flat.reshape(-1, 2)



# revision 5
# speedup vs baseline: 1.1499x; 1.0410x over previous
"""Trainium2 Bass kernel v6 for CubicalLayer gather_nd.

Problem: X[4096,4096] f32, indices[524288,2] int32 ->
         out[262144,2] f32, out.flat[k] = X[indices[k,0], indices[k,1]].

Strategy (8 NeuronCores, row-striped; ~65536 requests per core):
  - Host shards pairs by row stripe (core = r>>9) and phase class
    (ph = c%64, 64 classes padded to CLS=1152 slots) and precomputes the
    int16 256B-block index blk = (r%512)*64 + c//64 for each slot (the
    device does no index arithmetic; only the 1.2MB int16 index array is
    uploaded per pass instead of 4.7MB of int32 pairs).
  - Within a class, requests stay in natural order (measured on HW:
    sorted/rotated/interleaved orders are no faster than random for this
    SDMA gather -- it is descriptor-rate bound, not DRAM-row bound) and
    padding slots gather spread dummy blocks (recycled-duplicate padding
    concentrates addresses and measurably slows the gather).
  - Device: 32 SWDGE dma_gather ops per pass (2304 indices each; the Q7
    data scratch is 64KB so the int32 idx copy fits easily -- the old 1024
    "ring limit" was a chunk==ring-size wrap bug, so the descriptor ring
    carveout is raised to 4096 entries and chunks stay below it -- 256B
    blocks, round-robin over 4 queues), with
    single_packet=False so the SDMA engines batch many descriptors per
    packet instead of context-switching every 256B (measured ~20% faster);
    the vector engine extracts each class's phase element with static
    strided copies.
  - Host unshards: scatters per-core results back to original pair order.
"""

import numpy as np

import concourse.tile as tile
from concourse import bacc, mybir
from concourse.bass_utils import run_bass_kernel_spmd

H = 4096
W = 4096
N_IDX = 524288
NCORES = 8
P = 128

STRIPE_ROWS = H // NCORES  # 512
ELEM = 64  # f32 per gathered block (256B)
NPH = 64  # phase classes (c % 64)
CLS = 1152  # padded slots per class (9*128; binomial mean 1024, +4 sigma)
NPAD = NPH * CLS  # 73728
GCHUNK = 2304  # indices per dma_gather
NQ = 4  # SWDGE queues
NCHUNKS = NPAD // GCHUNK  # 72
COLS = NPAD // P  # 576
NBLK = STRIPE_ROWS * W // ELEM  # 32768 blocks per stripe
IDXW = NPAD // 16  # 4608


def build_kernel(reps=1):
    cg = GCHUNK // P  # 8 groups per chunk
    cls_g = CLS // P  # 9 groups per class

    nc = bacc.Bacc(
        "TRN2",
        target_bir_lowering=False,
        debug=False,
        num_devices=NCORES,
        num_swdge_queues=NQ,
        dynamic_dma_scratch_size=65536,
    )
    XS = nc.dram_tensor("XS", [STRIPE_ROWS, W], mybir.dt.float32, kind="ExternalInput")
    IDX = nc.dram_tensor("IDX", [P, IDXW], mybir.dt.int16, kind="ExternalInput")
    OUT = nc.dram_tensor("OUT", [P, COLS], mybir.dt.float32, kind="ExternalOutput")

    xs_rows = XS.ap().rearrange("h (a b) -> (h a) b", b=ELEM)  # [32768, 64]

    with tile.TileContext(nc) as tc:
        with (
            tc.tile_pool(name="ip", bufs=2) as i_pool,
            tc.tile_pool(name="gp", bufs=12) as g_pool,
            tc.tile_pool(name="op", bufs=1) as o_pool,
        ):
            vals = o_pool.tile([P, COLS], mybir.dt.float32)

            with tc.For_i(0, reps, 1):
                idx_sb = i_pool.tile([P, IDXW], mybir.dt.int16, tag="idx")
                nc.sync.dma_start(out=idx_sb[:, :], in_=IDX.ap())
                for c in range(NCHUNKS):
                    gsl = slice(c * (GCHUNK // 16), (c + 1) * (GCHUNK // 16))
                    g = g_pool.tile([P, cg, ELEM], mybir.dt.float32, tag="g")
                    nc.gpsimd.dma_gather(
                        out_ap=g[:, :, :],
                        in_ap=xs_rows,
                        idxs_ap=idx_sb[:, gsl],
                        num_idxs=GCHUNK,
                        num_idxs_reg=GCHUNK,
                        elem_size=ELEM,
                        queue_num=c % NQ,
                        single_packet=False,
                    )
                    # extract the phase element: the chunk's group range
                    # [8c, 8c+8) intersects classes (9 groups each) at
                    # static boundaries
                    g_lo = c * cg
                    while g_lo < (c + 1) * cg:
                        cls_idx = g_lo // cls_g
                        g_hi = min((cls_idx + 1) * cls_g, (c + 1) * cg)
                        nc.vector.tensor_copy(
                            out=vals[:, g_lo:g_hi],
                            in_=g[:, g_lo - c * cg : g_hi - c * cg, cls_idx],
                        )
                        g_lo = g_hi

            nc.sync.dma_start(out=OUT.ap(), in_=vals[:, :])
    nc.compile()
    return nc


_NC_CACHE = {}


def _get_nc():
    if "nc" not in _NC_CACHE:
        _NC_CACHE["nc"] = build_kernel()
    return _NC_CACHE["nc"]


def _route(indices):
    """Host-side shard: route pairs to (core, class) slots, build the int16
    block-index array per core and the slot->pair map."""
    r = indices[:, 0].astype(np.int64)
    c = indices[:, 1].astype(np.int64)
    core = r >> 9
    ph = c & (NPH - 1)
    blk = ((r & (STRIPE_ROWS - 1)) << 6) | (c >> 6)  # [0, 32768)
    key = core * NPH + ph  # 512 bins
    order = np.argsort(key, kind="stable")
    counts = np.bincount(key, minlength=NCORES * NPH)
    assert counts.max() <= CLS, f"class count {counts.max()} exceeds CLS={CLS}"
    starts = np.concatenate([[0], np.cumsum(counts)])

    # padding slots gather spread dummy blocks (uniform over the stripe);
    # measured: natural request order beats sorted/interleaved orders, and
    # recycled-duplicate padding concentrates addresses and slows SDMA
    pad_fill = ((np.arange(NPAD, dtype=np.int64) * 9973) & (NBLK - 1)).astype(np.int16)

    in_maps = []
    gather_pos = []
    for i in range(NCORES):
        idx_arr = pad_fill.copy()
        slot_all = []
        rows_all = []
        for p in range(NPH):
            b = i * NPH + p
            seg = order[starts[b] : starts[b + 1]]  # pair rows
            n = len(seg)
            base = p * CLS
            if n == 0:
                continue
            idx_arr[base : base + n] = blk[seg].astype(np.int16)
            slot_all.append(base + np.arange(n))
            rows_all.append(seg)
        iw = np.tile(idx_arr.reshape(NPAD // 16, 16).T, (8, 1))
        in_maps.append({"IDX": iw})
        gather_pos.append((np.concatenate(slot_all), np.concatenate(rows_all)))
    return in_maps, gather_pos


def kernel(X, indices):
    X = np.ascontiguousarray(np.asarray(X), dtype=np.float32)
    indices = np.asarray(indices, dtype=np.int32)
    nc = _get_nc()
    in_maps, gather_pos = _route(indices)
    for i in range(NCORES):
        in_maps[i]["XS"] = np.ascontiguousarray(
            X[i * STRIPE_ROWS : (i + 1) * STRIPE_ROWS]
        )
    res = run_bass_kernel_spmd(nc, in_maps, core_ids=list(range(NCORES)))
    out_flat = np.empty(N_IDX, np.float32)
    k = np.arange(NPAD)
    # routed slot k -> flat position in [P, COLS]: vals[k%128, k//128]
    land = (k % P) * COLS + k // P
    for i in range(NCORES):
        vals = res.results[i]["OUT"].reshape(-1)
        slots, rows = gather_pos[i]
        out_flat[rows] = vals[land[slots]]
    return out_flat.reshape(-1, 2)

